# revision 21
# baseline (speedup 1.0000x reference)
"""Trainium2 Bass kernel for nn_DetectionLoss (SSD-style detection loss).

Data-parallel over batch: 16 images, 8 cores, 2 images/core.

v3 design: fp16 (g,u)-layout IoU grid at DVE 2x; log-domain matching
(ls = ln(S)-ln(inter), argmax iou == argmin ls, pos <=> ls < ln3); coords
scaled 1/16 for fp16 range. The row-argmin one-hot is written in
(q, g, r) layout so each 128-col PE chunk spans 8 gts x 16 anchors; a
transpose-load + one matmul per chunk against a per-8-gt diagonal matrix
(static diag shipped from host x gt coords selected by tiny PE matmuls)
accumulates matched gt coords with no per-gt build work. Emission is
software-pipelined: image 1's grid overlaps image 0's PE matching; focal
+ mining run before DIoU so the matched coords are never waited on.
Hard negatives use a 2-level 16-bin threshold search; host combines
per-image scalars exactly as the reference.
"""
import sys

sys.path.insert(0, '/opt/trn_rl_repo')

import numpy as np
import concourse.bass as bass
import concourse.bacc as bacc
import concourse.mybir as mybir
from concourse.tile import TileContext
from concourse.bass_utils import run_bass_kernel_spmd
from concourse.masks import make_identity
from contextlib import ExitStack

Alu = mybir.AluOpType
Act = mybir.ActivationFunctionType
Ax = mybir.AxisListType
F32 = mybir.dt.float32
F16 = mybir.dt.float16
I32 = mybir.dt.int32

P = 128
A = 65536
G = 32
IMG = 2
NCORE = 8
COLS = 512          # anchor columns per partition
GU = G * COLS       # 16384 grid elems per partition
HSUP = COLS // 2    # supertile u-extent (256)
R = 16              # gt-expansion run length / diag block
QF = COLS // R      # 32
SCALE = 1.0 / 16.0
LNB = 1e-4          # bias inside ln()
LN3 = float(np.log(3.0))
NBIN = 8
NLEV = 3
NEG_POS_RATIO = 3.0


def _build_nc():
    nc = bacc.Bacc("TRN2", target_bir_lowering=False, debug=False)
    anch_d = nc.dram_tensor("anch", [P, COLS * 4], F32, kind="ExternalInput")
    bbox_d = nc.dram_tensor("bbox", [IMG, P, COLS * 4], F32, kind="ExternalInput")
    conf_d = nc.dram_tensor("conf", [IMG, P, COLS], F32, kind="ExternalInput")
    gt_d = nc.dram_tensor("gtb", [IMG, 1, G * 4], F32, kind="ExternalInput")
    m16c_d = nc.dram_tensor("m16c", [P, R * 4], F16, kind="ExternalInput")
    res_d = nc.dram_tensor("res", [IMG, 1, 8], F32, kind="ExternalOutput")

    v = nc.vector
    sc = nc.scalar
    pe = nc.tensor

    with TileContext(nc) as tc, ExitStack() as ctx:
        pool = ctx.enter_context(tc.tile_pool(name="main", bufs=1))
        pspool = ctx.enter_context(tc.tile_pool(name="ps", bufs=1, space="PSUM"))

        def T(name, cols, dt=F16, parts=P):
            return pool.tile([parts, cols], dt, name=name)

        # ---------------- big grid tiles ----------------
        lt = T("lt", GU)
        rb = T("rb", GU)
        ls = T("ls", GU)
        fb16 = T("fb16", GU)        # one-hot in (q, g, r) layout
        # ---------------- anchors ----------------
        a16 = T("a16", COLS * 4)            # planar (c4, col)
        areaA16 = T("areaA16", COLS)
        # ---------------- per-image inputs ----------------
        bbox_sb = T("bbox_sb", COLS * 4, F32)
        b16 = [T(f"b16_{i}", COLS * 4) for i in range(IMG)]
        conf_sb = [T(f"conf_sb{i}", COLS, F32) for i in range(IMG)]
        c16 = [T(f"c16_{i}", COLS) for i in range(IMG)]
        gtrow_sb = [T(f"gtrow_sb{i}", G * 4, F32, parts=1) for i in range(IMG)]
        gt_il = T("gt_il", G * 4, F32)
        gt_il16 = [T(f"gt_il16_{i}", G * 4) for i in range(IMG)]
        gtsb32 = [T(f"gtsb32_{i}", 4, F32, parts=G) for i in range(IMG)]
        gtsb16 = [T(f"gtsb16_{i}", 4, parts=G) for i in range(IMG)]
        gx1e = [T(f"gx1e{i}", COLS) for i in range(IMG)]
        gy1e = [T(f"gy1e{i}", COLS) for i in range(IMG)]
        gx2e = [T(f"gx2e{i}", COLS) for i in range(IMG)]
        gy2e = [T(f"gy2e{i}", COLS) for i in range(IMG)]
        sg_e = [T(f"sg_e{i}", COLS) for i in range(IMG)]
        gwh = T("gwh", G * 2)
        sg16 = T("sg16", G)
        colminb_e = T("colminb_e", COLS)
        # ---------------- matching outputs ----------------
        rowmin = T("rowmin", COLS)
        colpart = T("colpart", G)
        forced = T("forced", COLS)
        pos = [T(f"pos{i}", COLS) for i in range(IMG)]
        npp = [T(f"npp{i}", 1, F32) for i in range(IMG)]
        # ---------------- matched coords / PE ----------------
        m16diag = T("m16diag", R * 4)       # [128, (r',c)] = delta(p%16==r')
        E4 = [T(f"E4_{i}", P, parts=G) for i in range(4)]
        Dmat = [T(f"Dmat{i}", R * 4) for i in range(4)]
        rowgt = [T(f"rowgt{i}", 4) for i in range(4)]
        tsb = T("tsb", P * 4)
        m16 = [T(f"m16_{i}", COLS * 4) for i in range(IMG)]
        cmin_c = pool.tile([G, 1], F32, name="cmin_c")
        # ---------------- DIoU / focal scratch ----------------
        dw = T("dw", COLS * 2)
        dw2 = T("dw2", COLS * 2)
        s0 = T("s0", COLS)
        s1 = T("s1", COLS)
        s2 = T("s2", COLS)
        s3 = T("s3", COLS)
        s4 = T("s4", COLS)
        u32a = T("u32a", COLS, F32)
        u32b = T("u32b", COLS, F32)
        rec16 = T("rec16", COLS)
        cl16 = T("cl16", COLS)
        nv16 = [T(f"nv16_{i}", COLS) for i in range(IMG)]
        sink16 = T("sink16", COLS)
        # ---------------- small f32 machinery ----------------
        ident = T("ident", P, F32)
        ident16 = T("ident16", P)
        ones_col = T("ones_col", 1, F32)
        ones_row = T("ones_row", P, F32, parts=1)
        lnb_t = T("lnb_t", 1, F32)
        cm_row = T("cm_row", G, F32, parts=1)
        cmb = T("cmb", G, F32)
        mx_row = T("mx_row", P, F32, parts=1)
        locsum_pp = T("locsum_pp", 1, F32)
        possum_pp = T("possum_pp", 1, F32)
        cnt_pp = T("cnt_pp", 1, F32)
        sum_pp = T("sum_pp", 1, F32)
        maxv_pp = T("maxv_pp", 1, F32)
        maxvb = T("maxvb", 1, F32)
        w1c = T("w1c", 1, F32)
        tau_b = T("tau_b", 1, F32)
        stack = T("stack", 4, F32)
        iota_i = pool.tile([P, NBIN], I32, name="iota_i")
        iota_f = T("iota_f", NBIN, F32)
        thr = T("thr", NBIN, F32)
        cge = T("cge", NBIN, F32)
        wl = [T(f"wl{l}", 1, F32) for l in range(NLEV)]
        lo_b = [T(f"lo_b{l}", 1, F32) for l in range(NLEV)]
        cget = T("cget", NBIN, F32, parts=1)
        gek = T("gek", NBIN, F32, parts=1)
        scnt = T("scnt", 1, F32, parts=1)
        lo_new = T("lo_new", 1, F32, parts=1)
        tau = [T(f"tau{l}", 1, F32, parts=1) for l in range(NLEV)]
        maxv1 = T("maxv1", 1, F32, parts=1)
        npos1 = T("npos1", 1, F32, parts=1)
        k1 = T("k1", 1, F32, parts=1)
        k2 = T("k2", 1, F32, parts=1)
        kk = T("kk", 1, F32, parts=1)
        res_sb = T("res_sb", 8, F32, parts=1)

        # ---------------- initial DMAs ----------------
        nc.sync.dma_start(bbox_sb[:], anch_d[:])   # anchors stage via bbox_sb
        nc.sync.dma_start(m16diag[:], m16c_d[:])
        for b in range(IMG):
            nc.sync.dma_start(gtrow_sb[b][:], gt_d[b])
            gt2d = gt_d[b].rearrange("q (g c) -> (q g) c", c=4)
            nc.sync.dma_start(gtsb32[b][:], gt2d)
            nc.scalar.dma_start(conf_sb[b][:], conf_d[b])

        # ---------------- constants ----------------
        v.memset(ones_col[:], 1.0)
        v.memset(ones_row[:], 1.0)
        v.memset(lnb_t[:], LNB)
        make_identity(nc, ident[:])
        v.tensor_scalar(ident16[:], ident[:], 1.0, None, Alu.mult)
        nc.gpsimd.iota(iota_i[:], pattern=[[1, NBIN]], base=0, channel_multiplier=0)
        v.tensor_copy(iota_f[:], iota_i[:])
        # E4[gblk][g, (g_l, r)] = delta(g == 8*gblk + g_l), static
        for gblk in range(4):
            ev = ident16[0:G, 8 * gblk:8 * gblk + 8] \
                .unsqueeze(2).to_broadcast([G, 8, R])
            sc.activation(E4[gblk][:].rearrange("p (e r) -> p e r", r=R), ev,
                          Act.Copy)

        # a16 planar = anch interleaved, scaled 1/16 (staged in bbox_sb)
        a_il = bbox_sb[:].rearrange("p (n c) -> p c n", c=4)
        a16v = a16[:].rearrange("p (c n) -> p c n", c=4)
        sc.activation(a16v, a_il, Act.Copy, scale=SCALE)
        v.tensor_tensor(out=dw[:], in0=a16[:, 2 * COLS:4 * COLS],
                        in1=a16[:, 0:2 * COLS], op=Alu.subtract)
        v.tensor_tensor(out=areaA16[:], in0=dw[:, 0:COLS],
                        in1=dw[:, COLS:], op=Alu.mult)

        def pbcast(dst, src_row, n):
            bc_ps = pspool.tile([P, G], F32, name="bc_ps", tag="pss")
            nc.tensor.matmul(bc_ps[:, 0:n], ones_row[:, 0:P], src_row)
            v.tensor_copy(dst, bc_ps[:, 0:n])

        def pbcast_wide(dst, src_row, n):
            bc_ps = pspool.tile([P, G * 4], F32, name="bcw_ps", tag="pssw")
            nc.tensor.matmul(bc_ps[:, 0:n], ones_row[:, 0:P], src_row)
            v.tensor_copy(dst, bc_ps[:, 0:n])

        # ================= phase functions =================
        def prep(b):
            nc.scalar.dma_start(bbox_sb[:], bbox_d[b])
            pbcast_wide(gt_il[:], gtrow_sb[b][:], G * 4)
            v.tensor_scalar(gt_il16[b][:], gt_il[:], SCALE, None, Alu.mult)
            v.tensor_scalar(gtsb16[b][:], gtsb32[b][:], SCALE, None, Alu.mult)
            gil = gt_il16[b][:].rearrange("p (g c) -> p g c", c=4)
            for c, dst in ((0, gx1e[b]), (1, gy1e[b]), (2, gx2e[b]), (3, gy2e[b])):
                src = gil[:, :, c:c + 1].to_broadcast([P, G, R])
                sc.activation(dst[:].rearrange("p (g r) -> p g r", r=R), src,
                              Act.Copy)
            v.tensor_tensor(out=gwh[:, 0:G], in0=gil[:, :, 2:3].squeeze(2),
                            in1=gil[:, :, 0:1].squeeze(2), op=Alu.subtract)
            v.tensor_tensor(out=gwh[:, G:], in0=gil[:, :, 3:4].squeeze(2),
                            in1=gil[:, :, 1:2].squeeze(2), op=Alu.subtract)
            v.tensor_tensor(out=sg16[:], in0=gwh[:, 0:G], in1=gwh[:, G:],
                            op=Alu.mult)
            sgsrc = sg16[:].unsqueeze(2).to_broadcast([P, G, R])
            sc.activation(sg_e[b][:].rearrange("p (g r) -> p g r", r=R), sgsrc,
                          Act.Copy)
            bb_il = bbox_sb[:].rearrange("p (n c) -> p c n", c=4)
            b16v = b16[b][:].rearrange("p (c n) -> p c n", c=4)
            sc.activation(b16v, bb_il, Act.Copy, scale=SCALE)
            sc.activation(c16[b][:], conf_sb[b][:], Act.Copy)

        def grid(b):
            # quarter supertiles (u=128) ping-ponging between halves of
            # lt/rb so DVE compute of tile s+1 overlaps scalar Ln of tile s
            US = COLS // 4          # 128 anchor cols per supertile
            QS = US // R            # 8
            SZ = 2 * G * US         # lt/rb elems per supertile (8192)
            HG = G * US             # grid elems per supertile (4096)
            a16r = a16[:].rearrange("p (c n) -> p c n", c=4)
            glo = (gx1e[b], gy1e[b])
            ghi = (gx2e[b], gy2e[b])
            for s in range(4):
                usl = slice(s * US, (s + 1) * US)
                lth = lt[:, (s % 2) * SZ:(s % 2 + 1) * SZ]
                rbh = rb[:, (s % 2) * SZ:(s % 2 + 1) * SZ]
                lt4 = lth.rearrange("p (c g q r) -> p c g q r", c=2, g=G,
                                    q=QS, r=R)
                rb4 = rbh.rearrange("p (c g q r) -> p c g q r", c=2, g=G,
                                    q=QS, r=R)
                for k in range(2):
                    aslab = a16r[:, k:k + 1, usl].squeeze(1) \
                        .rearrange("p (q r) -> p q r", r=R) \
                        .unsqueeze(1).to_broadcast([P, G, QS, R])
                    gslab = glo[k][:].rearrange("p (g r) -> p g r", r=R) \
                        .unsqueeze(2).to_broadcast([P, G, QS, R])
                    v.tensor_tensor(out=lt4[:, k:k + 1].squeeze(1), in0=aslab,
                                    in1=gslab, op=Alu.max)
                    aslab2 = a16r[:, k + 2:k + 3, usl].squeeze(1) \
                        .rearrange("p (q r) -> p q r", r=R) \
                        .unsqueeze(1).to_broadcast([P, G, QS, R])
                    gslab2 = ghi[k][:].rearrange("p (g r) -> p g r", r=R) \
                        .unsqueeze(2).to_broadcast([P, G, QS, R])
                    v.tensor_tensor(out=rb4[:, k:k + 1].squeeze(1), in0=aslab2,
                                    in1=gslab2, op=Alu.min)
                v.tensor_tensor(out=lth, in0=rbh, in1=lth, op=Alu.subtract)
                v.tensor_scalar(lth, lth, 0.0, None, Alu.max)
                v.tensor_tensor(out=rbh[:, 0:HG], in0=lth[:, 0:HG],
                                in1=lth[:, HG:], op=Alu.mult)
                areab = areaA16[:, usl].rearrange("p (q r) -> p q r", r=R) \
                    .unsqueeze(1).to_broadcast([P, G, QS, R])
                sgb = sg_e[b][:].rearrange("p (g r) -> p g r", r=R) \
                    .unsqueeze(2).to_broadcast([P, G, QS, R])
                csv = rbh[:, HG:].rearrange("p (g q r) -> p g q r", q=QS, r=R)
                v.tensor_tensor(out=csv, in0=areab, in1=sgb, op=Alu.add)
                sc.activation(rbh[:, 0:HG], rbh[:, 0:HG], Act.Ln,
                              bias=lnb_t[0:P, :])
                sc.activation(rbh[:, HG:], rbh[:, HG:], Act.Ln,
                              bias=lnb_t[0:P, :])
                lsv = ls[:].rearrange("p (g u) -> p g u", u=COLS)[:, :, usl]
                v.tensor_tensor(out=lsv,
                                in0=rbh[:, HG:].rearrange(
                                    "p (g u) -> p g u", u=US),
                                in1=rbh[:, 0:HG].rearrange(
                                    "p (g u) -> p g u", u=US),
                                op=Alu.subtract)

        def match(b):
            # rowmin tree over g
            v.tensor_tensor(out=lt[:, 0:8192], in0=ls[:, 0:8192],
                            in1=ls[:, 8192:], op=Alu.min)
            for n in (4096, 2048, 1024, 512):
                v.tensor_tensor(out=lt[:, 0:n], in0=lt[:, 0:n],
                                in1=lt[:, n:2 * n], op=Alu.min)
            v.tensor_copy(rowmin[:], lt[:, 0:512])
            # one-hot, written in (q, g, r) layout for the PE chunks
            lsg4 = ls[:].rearrange("p (g q r) -> p g q r", q=QF, r=R)
            fbq = fb16[:].rearrange("p (q g r) -> p g q r", q=QF, g=G, r=R)
            rmb = rowmin[:].rearrange("p (q r) -> p q r", r=R) \
                .unsqueeze(1).to_broadcast([P, G, QF, R])
            v.tensor_tensor(out=fbq, in0=lsg4, in1=rmb, op=Alu.is_equal)
            # colmin partial tree over u per g
            lsg = ls[:].rearrange("p (g u) -> p g u", u=COLS)
            n = 256
            v.tensor_tensor(out=rb[:, 0:G * n].rearrange("p (g u) -> p g u", u=n),
                            in0=lsg[:, :, 0:n], in1=lsg[:, :, n:2 * n],
                            op=Alu.min)
            src_off = 0
            while n > 1:
                m = n // 2
                sv = rb[:, src_off:src_off + G * n].rearrange(
                    "p (g u) -> p g u", u=n)
                dst_off = G * n if src_off == 0 else 0
                dv = rb[:, dst_off:dst_off + G * m].rearrange(
                    "p (g u) -> p g u", u=m)
                v.tensor_tensor(out=dv, in0=sv[:, :, 0:m], in1=sv[:, :, m:n],
                                op=Alu.min)
                src_off = dst_off
                n = m
            v.tensor_copy(colpart[:], rb[:, src_off:src_off + G])
            # colmin finalize + broadcast
            ct_ps = pspool.tile([G, P], F16, name="ct_ps", tag="pss")
            pe.transpose(ct_ps[:], colpart[:], ident16[:])
            v.tensor_reduce(out=cmin_c[:], in_=ct_ps[:], axis=Ax.X, op=Alu.min)
            cm_ps = pspool.tile([1, G], F32, name="cm_ps", tag="pssw")
            pe.transpose(cm_ps[:], cmin_c[:], ident[:G, :G])
            v.tensor_copy(cm_row[:], cm_ps[:])
            pbcast(cmb[:], cm_row[:], G)
            cmbe = colminb_e[:].rearrange("p (g r) -> p g r", r=R)
            sc.activation(cmbe, cmb[:].unsqueeze(2).to_broadcast([P, G, R]),
                          Act.Copy)
            # forced = any_g(ls == colmin_g)
            cmbb = colminb_e[:].rearrange("p (g r) -> p g r", r=R) \
                .unsqueeze(2).to_broadcast([P, G, QF, R])
            lt4f = lt[:].rearrange("p (g q r) -> p g q r", q=QF, r=R)
            v.tensor_tensor(out=lt4f, in0=lsg4, in1=cmbb, op=Alu.is_equal)
            v.tensor_tensor(out=rb[:, 0:8192], in0=lt[:, 0:8192],
                            in1=lt[:, 8192:], op=Alu.max)
            for n in (4096, 2048, 1024, 512):
                v.tensor_tensor(out=rb[:, 0:n], in0=rb[:, 0:n],
                                in1=rb[:, n:2 * n], op=Alu.max)
            v.tensor_copy(forced[:], rb[:, 0:512])
            # pos / n_pos
            v.tensor_scalar(pos[b][:], rowmin[:], LN3, None, Alu.is_lt)
            v.tensor_tensor(out=pos[b][:], in0=pos[b][:], in1=forced[:],
                            op=Alu.max)
            v.tensor_reduce(out=npp[b][:], in_=pos[b][:], axis=Ax.X, op=Alu.add)

        def matched_coords(b):
            # Dmat[gblk][(g_l,r), (r',c)] = delta(r==r') * gt[8*gblk+g_l, c]
            for gblk in range(4):
                rg_ps = pspool.tile([P, G], F32, name="rg_ps", tag="pss")
                nc.tensor.matmul(rg_ps[:, 0:4], E4[gblk][:], gtsb16[b][:])
                v.tensor_copy(rowgt[gblk][:], rg_ps[:, 0:4])
                rgb = rowgt[gblk][:].unsqueeze(1).to_broadcast([P, R, 4])
                v.tensor_tensor(out=Dmat[gblk][:].rearrange(
                    "p (r c) -> p r c", c=4), in0=m16diag[:].rearrange(
                    "p (r c) -> p r c", c=4), in1=rgb, op=Alu.mult)
            mt_ps = [pspool.tile([P, P * 4], F32, name=f"mt{u}", tag=f"mt{u}")
                     for u in range(4)]
            for q in range(QF):
                tp_ps = pspool.tile([P, P * 4], F16, name=f"tp{q % 2}",
                                    tag=f"tp{q % 2}")
                for gblk in range(4):
                    pe.transpose(tp_ps[:, gblk * P:(gblk + 1) * P],
                                 fb16[:, q * COLS + gblk * P:
                                      q * COLS + (gblk + 1) * P],
                                 ident16[:])
                sc.activation(tsb[:], tp_ps[:], Act.Copy)
                mk = mt_ps[q // 8]
                csl = slice((q % 8) * 64, (q % 8) * 64 + 64)
                for gblk in range(4):
                    nc.tensor.matmul(mk[:, csl], tsb[:, gblk * P:(gblk + 1) * P],
                                     Dmat[gblk][:], start=(gblk == 0),
                                     stop=(gblk == 3))
            m16v = m16[b][:].rearrange("p (c u) -> p c u", c=4)
            for k in range(4):
                src = mt_ps[k][:].rearrange("p (q r c) -> p c q r", r=R, c=4)
                dst = m16v[:, :, k * P:(k + 1) * P].rearrange(
                    "p c (q r) -> p c q r", r=R)
                sc.activation(dst, src, Act.Copy)

        def diou(b):
            bl = b16[b][:, 0:2 * COLS]
            bh = b16[b][:, 2 * COLS:]
            ml = m16[b][:, 0:2 * COLS]
            mh = m16[b][:, 2 * COLS:]
            v.tensor_tensor(out=dw[:], in0=bl, in1=ml, op=Alu.max)
            v.tensor_tensor(out=dw2[:], in0=bh, in1=mh, op=Alu.min)
            v.tensor_tensor(out=dw[:], in0=dw2[:], in1=dw[:], op=Alu.subtract)
            v.tensor_scalar(dw[:], dw[:], 0.0, 100.0, Alu.max, Alu.min)
            v.tensor_tensor(out=s0[:], in0=dw[:, 0:COLS], in1=dw[:, COLS:],
                            op=Alu.mult)                       # inter
            v.tensor_tensor(out=dw2[:], in0=bh, in1=bl, op=Alu.subtract)
            v.tensor_tensor(out=s1[:], in0=dw2[:, 0:COLS], in1=dw2[:, COLS:],
                            op=Alu.mult)                       # areaP
            v.tensor_tensor(out=dw2[:], in0=mh, in1=ml, op=Alu.subtract)
            v.tensor_tensor(out=s2[:], in0=dw2[:, 0:COLS], in1=dw2[:, COLS:],
                            op=Alu.mult)                       # areaM
            v.tensor_tensor(out=s1[:], in0=s1[:], in1=s2[:], op=Alu.add)
            v.tensor_tensor(out=s1[:], in0=s1[:], in1=s0[:], op=Alu.subtract)
            v.tensor_scalar(u32a[:], s1[:], 1e6, 0.01, Alu.min, Alu.max)
            v.reciprocal_approx_fast(out=u32b[:], in_=u32a[:])
            v.tensor_copy(rec16[:], u32b[:])
            v.tensor_tensor(out=s0[:], in0=s0[:], in1=rec16[:], op=Alu.mult)
            v.tensor_tensor(out=dw[:], in0=bl, in1=ml, op=Alu.min)
            v.tensor_tensor(out=dw2[:], in0=bh, in1=mh, op=Alu.max)
            v.tensor_tensor(out=dw[:], in0=dw2[:], in1=dw[:], op=Alu.subtract)
            v.tensor_tensor(out=dw[:], in0=dw[:], in1=dw[:], op=Alu.mult)
            v.tensor_tensor(out=s2[:], in0=dw[:, 0:COLS], in1=dw[:, COLS:],
                            op=Alu.add)                        # c2
            v.tensor_scalar(u32a[:], s2[:], 1e6, 0.01, Alu.min, Alu.max)
            v.reciprocal_approx_fast(out=u32b[:], in_=u32a[:])
            v.tensor_copy(rec16[:], u32b[:])
            v.tensor_tensor(out=dw[:], in0=bl, in1=bh, op=Alu.add)
            v.tensor_tensor(out=dw2[:], in0=ml, in1=mh, op=Alu.add)
            v.tensor_tensor(out=dw[:], in0=dw[:], in1=dw2[:], op=Alu.subtract)
            v.tensor_tensor(out=dw[:], in0=dw[:], in1=dw[:], op=Alu.mult)
            v.tensor_tensor(out=s3[:], in0=dw[:, 0:COLS], in1=dw[:, COLS:],
                            op=Alu.add)                        # 4*d2
            v.tensor_tensor(out=s3[:], in0=s3[:], in1=rec16[:], op=Alu.mult)
            v.tensor_scalar(s3[:], s3[:], 0.25, None, Alu.mult)
            v.tensor_scalar(s0[:], s0[:], -1.0, 1.0, Alu.mult, Alu.add)
            v.tensor_tensor(out=s3[:], in0=s3[:], in1=s0[:], op=Alu.add)
            v.tensor_scalar(s3[:], s3[:], 100.0, None, Alu.min)
            v.tensor_tensor(out=s4[:], in0=s3[:], in1=pos[b][:], op=Alu.mult)
            v.tensor_reduce(out=locsum_pp[:], in_=s4[:], axis=Ax.X, op=Alu.add)

        def focal_mining(b):
            sc.activation(s1[:], c16[b][:], Act.Exp)
            sc.activation(s1[:], s1[:], Act.Ln, bias=1.0)
            v.tensor_tensor(out=s0[:], in0=c16[b][:], in1=pos[b][:], op=Alu.mult)
            v.tensor_tensor(out=s2[:], in0=s1[:], in1=s0[:], op=Alu.subtract)
            v.tensor_tensor(out=s0[:], in0=c16[b][:], in1=s0[:], op=Alu.subtract)
            v.tensor_tensor(out=s0[:], in0=s0[:], in1=s1[:], op=Alu.subtract)
            sc.activation(s0[:], s0[:], Act.Exp, scale=2.0)
            v.tensor_tensor(out=cl16[:], in0=s2[:], in1=s0[:], op=Alu.mult)
            v.tensor_scalar(s3[:], pos[b][:], -0.5, 0.75, Alu.mult, Alu.add)
            v.tensor_tensor(out=cl16[:], in0=cl16[:], in1=s3[:], op=Alu.mult)
            v.tensor_scalar(cl16[:], cl16[:], 100.0, None, Alu.min)
            v.tensor_tensor(out=s4[:], in0=cl16[:], in1=pos[b][:], op=Alu.mult)
            v.tensor_reduce(out=possum_pp[:], in_=s4[:], axis=Ax.X, op=Alu.add)
            v.tensor_tensor(out=nv16[b][:], in0=cl16[:], in1=s4[:],
                            op=Alu.subtract)
            # mining
            v.tensor_reduce(out=maxv_pp[:], in_=nv16[b][:], axis=Ax.X, op=Alu.max)
            mx_ps = pspool.tile([1, P], F32, name="mx_ps", tag="pss")
            pe.transpose(mx_ps[:], maxv_pp[:], ident[:])
            v.tensor_copy(mx_row[:], mx_ps[:])
            v.tensor_reduce(out=maxv1[:], in_=mx_row[:], axis=Ax.X, op=Alu.max)
            np_ps = pspool.tile([1, 1], F32, name="np_ps", tag="pss")
            nc.tensor.matmul(np_ps[:], ones_col[:], npp[b][:])
            v.tensor_copy(npos1[:], np_ps[:])
            v.tensor_scalar(k1[:], npos1[:], NEG_POS_RATIO, None, Alu.mult)
            v.tensor_scalar(k2[:], npos1[:], -1.0, float(A), Alu.mult, Alu.add)
            v.tensor_tensor(out=kk[:], in0=k1[:], in1=k2[:], op=Alu.min)
            pbcast(maxvb[:], maxv1[:], 1)
            v.tensor_scalar(w1c[:], maxvb[:], 1.0 / NBIN, None, Alu.mult)
            for lev in range(NLEV):
                if lev == 0:
                    v.tensor_copy(wl[0][:], w1c[:])
                    v.tensor_scalar(thr[:], iota_f[:], wl[0][:], None, Alu.mult)
                else:
                    v.tensor_scalar(wl[lev][:], wl[lev - 1][:], 1.0 / NBIN, None,
                                    Alu.mult)
                    v.tensor_scalar(thr[:], iota_f[:], wl[lev][:],
                                    lo_b[lev - 1][:], Alu.mult, Alu.add)
                for bn in range(NBIN):
                    v.tensor_scalar(sink16[:], nv16[b][:], thr[:, bn:bn + 1], 0.0,
                                    Alu.is_gt, Alu.add,
                                    accum_out=cge[:, bn:bn + 1])
                cg_ps = pspool.tile([1, NBIN], F32, name="cg_ps", tag="pss")
                nc.tensor.matmul(cg_ps[:], ones_col[:], cge[:])
                v.tensor_copy(cget[:], cg_ps[:])
                v.tensor_scalar(gek[:], cget[:], kk[:], None, Alu.is_ge)
                v.tensor_reduce(out=scnt[:], in_=gek[:], axis=Ax.X, op=Alu.add)
                v.tensor_scalar(lo_new[:], scnt[:], 1.0, wl[lev][0:1, :],
                                Alu.subtract, Alu.mult)
                v.tensor_scalar(tau[lev][:], scnt[:], wl[lev][0:1, :], None,
                                Alu.mult)
                if lev > 0:
                    v.tensor_tensor(out=lo_new[:], in0=lo_new[:],
                                    in1=lo_b[lev - 1][0:1, :], op=Alu.add)
                    v.tensor_tensor(out=tau[lev][:], in0=tau[lev][:],
                                    in1=lo_b[lev - 1][0:1, :], op=Alu.add)
                pbcast(lo_b[lev][:], lo_new[:], 1)
            pbcast(tau_b[:], tau[NLEV - 1][:], 1)
            v.tensor_scalar(s4[:], nv16[b][:], tau_b[:], 0.0, Alu.is_gt,
                            Alu.add, accum_out=cnt_pp[:])
            v.tensor_tensor(out=s3[:], in0=nv16[b][:], in1=s4[:], op=Alu.mult)
            v.tensor_reduce(out=sum_pp[:], in_=s3[:], axis=Ax.X, op=Alu.add)

        def gather(b):
            v.tensor_copy(stack[:, 0:1], npp[b][:])
            v.tensor_copy(stack[:, 1:2], locsum_pp[:])
            v.tensor_copy(stack[:, 2:3], possum_pp[:])
            v.tensor_copy(stack[:, 3:4], cnt_pp[:])
            st_ps = pspool.tile([1, 4], F32, name="st_ps", tag="pss")
            nc.tensor.matmul(st_ps[:], ones_col[:], stack[:])
            sm_ps = pspool.tile([1, 1], F32, name="sm_ps", tag="pss")
            nc.tensor.matmul(sm_ps[:], ones_col[:], sum_pp[:])
            v.tensor_copy(res_sb[:, 0:4], st_ps[:])
            v.tensor_copy(res_sb[:, 4:5], sm_ps[:])
            v.tensor_copy(res_sb[:, 5:6], tau[NLEV - 1][:])
            v.tensor_copy(res_sb[:, 6:7], maxv1[:])
            v.tensor_copy(res_sb[:, 7:8], kk[:])
            nc.sync.dma_start(res_d[b], res_sb[:])

        # ================= pipelined emission =================
        prep(0)
        grid(0)
        match(0)
        prep(1)
        grid(1)
        matched_coords(0)
        match(1)
        matched_coords(1)
        focal_mining(0)
        diou(0)
        gather(0)
        focal_mining(1)
        diou(1)
        gather(1)

    nc.compile()
    return nc


_NC_CACHE = None


def _get_nc():
    global _NC_CACHE
    if _NC_CACHE is None:
        _NC_CACHE = _build_nc()
    return _NC_CACHE


def _m16_const():
    # [128, (r', c)] = delta(p % 16 == r'), fp16
    m = np.zeros((P, R, 4), dtype=np.float16)
    for p in range(P):
        m[p, p % R, :] = 1.0
    return np.ascontiguousarray(m.reshape(P, R * 4))


def _make_in_maps(inputs):
    bbox_pred = np.asarray(inputs["bbox_pred"])
    conf_pred = np.asarray(inputs["conf_pred"])
    anchors = np.asarray(inputs["anchors"])
    gt_boxes = np.asarray(inputs["gt_boxes"])
    anch_h = np.ascontiguousarray(anchors.reshape(P, COLS * 4), dtype=np.float32)
    m16c = _m16_const()
    in_maps = []
    for i in range(NCORE):
        bsl = slice(IMG * i, IMG * (i + 1))
        in_maps.append({
            "anch": anch_h,
            "m16c": m16c,
            "bbox": np.ascontiguousarray(
                bbox_pred[bsl].reshape(IMG, P, COLS * 4), dtype=np.float32),
            "conf": np.ascontiguousarray(
                conf_pred[bsl].reshape(IMG, P, COLS), dtype=np.float32),
            "gtb": np.ascontiguousarray(
                gt_boxes[bsl].reshape(IMG, 1, G * 4), dtype=np.float32),
        })
    return in_maps


def kernel(bbox_pred, conf_pred, anchors, gt_boxes):
    nc = _get_nc()
    in_maps = _make_in_maps(dict(bbox_pred=bbox_pred, conf_pred=conf_pred,
                                 anchors=anchors, gt_boxes=gt_boxes))
    out = run_bass_kernel_spmd(nc, in_maps, core_ids=list(range(NCORE)))

    loc_total = np.float32(0.0)
    conf_total = np.float32(0.0)
    npos_total = np.float32(0.0)
    for i in range(NCORE):
        res = out.results[i]["res"]  # [IMG, 1, 8]
        for b in range(IMG):
            npos, locsum, possum, cnt_gt, sum_gt, tau_hi, maxv, kdev = \
                [np.float32(x) for x in res[b, 0, :8]]
            k = np.float32(min(NEG_POS_RATIO * npos, A - npos))
            wl_last = np.float32(maxv / NBIN ** NLEV)
            rem = max(np.float32(0.0), np.float32(k - cnt_gt))
            neg = np.float32(sum_gt + rem * (tau_hi - wl_last * np.float32(0.5)))
            loc_total = np.float32(loc_total + locsum)
            conf_total = np.float32(conf_total + possum + neg)
            npos_total = np.float32(npos_total + npos)
    num_pos = np.float32(max(1.0, npos_total))
    loc_loss = np.float32(loc_total / num_pos)
    conf_loss = np.float32(conf_total / num_pos)
    return (np.float32(loc_loss + conf_loss), conf_loss, loc_loss)
